# revision 24
# baseline (speedup 1.0000x reference)
"""DA-Block (dual attention: PAM + CAM) Trainium2 Bass kernel.

Sharding: data-parallel over batch B (2 groups of 4 cores); within a group,
PAM's N x N attention is sharded row-wise over query voxels (1024 per core).
The small C x C CAM affinity is computed replicated on every core.

Per-core math (batch b, query chunk Q of 1024 voxels, N=4096, C=256, CQK=32):
  q = wq @ x[:, Q]            (32, 1024)
  k = wk @ x                  (32, 4096)
  vT = x^T @ wv^T             (4096, 256)   [no bias; bias folded into x2tc]
  eT[j, i] = sum_o k[o,j] q[o,i]            (PSUM, streamed over i-blocks)
  sT = exp(eT)                              (no max-sub; |e| < ~45 is fp32-safe)
  pam'[i, c'] = sum_j sT[j,i] * [vT | 1][j, c']   (c'=256 column gives denom d_i)
  camE = x @ x^T (replicated, fp32r), attn_c = softmax via exp(rowmin - e)
  camT[n, c] = sum_j attn_c[c,j] x[j, n]  for n in Q
  base[n, c] = gamma_c * camT[n, c] + (2 * xT[n, c] + gamma_p * bv[c])
  out[n, c] = gamma_p * pam'[n, c] / d_n + base[n, c]
Host gathers the (1024, 256) per-core outputs, transposes, reshapes.

Matmul dtypes: bf16 for projections and PAM attention (errors enter linearly
or through a ~0.03-abs energy perturbation that averages out in the softmax
mean); fp32r for the CAM Gram matrix, whose +-4300 energies would amplify
bf16 rounding through exp catastrophically.
"""

import sys

import numpy as np

for _p in ("/opt/trn_rl_repo", "/root/.axon_site/_ro/trn_rl_repo"):
    if _p not in sys.path:
        sys.path.append(_p)

B, C, CQK = 2, 256, 32
N = 4096  # 16*16*16
NCORES = 8
GROUP = 4  # cores per batch
CHUNK = N // GROUP  # 1024
IBLK = 256  # i-block (query) streaming width
NIB = CHUNK // IBLK  # 4
NJT = N // 128  # 32 j-tiles
CV = C + 2  # vT padded with two ones-columns (even free-dim for PSUM dst)


def _build(gp: float, gc: float):
    """Build the per-core Bass kernel with gamma values baked as immediates."""
    import concourse.bacc as bacc
    import concourse.mybir as mybir
    from concourse import tile

    f32 = mybir.dt.float32
    f32r = mybir.dt.float32r
    bf16 = mybir.dt.bfloat16
    Alu = mybir.AluOpType
    Act = mybir.ActivationFunctionType

    nc = bacc.Bacc("TRN2", target_bir_lowering=False, debug=False, num_devices=NCORES)

    xf = nc.dram_tensor("xf", [C, N], bf16, kind="ExternalInput").ap()
    xfc = nc.dram_tensor("xfc", [C, CHUNK], bf16, kind="ExternalInput").ap()
    xt = nc.dram_tensor("xt", [N, C], f32r, kind="ExternalInput").ap()
    x2tc = nc.dram_tensor("x2tc", [CHUNK, C], f32, kind="ExternalInput").ap()
    wqt = nc.dram_tensor("wqt", [C, CQK], bf16, kind="ExternalInput").ap()
    wkt = nc.dram_tensor("wkt", [C, CQK], bf16, kind="ExternalInput").ap()
    wvt = nc.dram_tensor("wvt", [C, C], bf16, kind="ExternalInput").ap()
    bq = nc.dram_tensor("bq", [CQK, 1], f32, kind="ExternalInput").ap()
    bk = nc.dram_tensor("bk", [CQK, 1], f32, kind="ExternalInput").ap()
    ident = nc.dram_tensor("ident", [128, 128], f32, kind="ExternalInput").ap()
    out = nc.dram_tensor("out", [CHUNK, C], f32, kind="ExternalOutput").ap()

    with tile.TileContext(nc) as tc:
        with (
            tc.tile_pool(name="singles", bufs=1) as sg,
            tc.tile_pool(name="sp", bufs=3) as sp,
            tc.tile_pool(name="misc", bufs=4) as misc,
            tc.tile_pool(name="ps", bufs=1, space="PSUM") as ps,
        ):
            # ---- constant / persistent SBUF tiles + DMAs ----
            # sync queue: weights, xfc, xf (feeds the early projections)
            # gpsimd queue: xt, x2tc (needed only by the later CAM phase)
            wqt0 = sg.tile([128, CQK], bf16, name="wqt0")
            wqt1 = sg.tile([128, CQK], bf16, name="wqt1")
            wkt0 = sg.tile([128, CQK], bf16, name="wkt0")
            wkt1 = sg.tile([128, CQK], bf16, name="wkt1")
            wvt0 = sg.tile([128, C], bf16, name="wvt0")
            wvt1 = sg.tile([128, C], bf16, name="wvt1")
            bq_s = sg.tile([CQK, 1], f32, name="bq_s")
            bk_s = sg.tile([CQK, 1], f32, name="bk_s")
            ident_s = sg.tile([128, 128], f32, name="ident_s")
            nc.sync.dma_start(out=wqt0[:], in_=wqt[0:128, :])
            nc.sync.dma_start(out=wqt1[:], in_=wqt[128:256, :])
            nc.sync.dma_start(out=wkt0[:], in_=wkt[0:128, :])
            nc.sync.dma_start(out=wkt1[:], in_=wkt[128:256, :])
            nc.sync.dma_start(out=bq_s[:], in_=bq[:])
            nc.sync.dma_start(out=bk_s[:], in_=bk[:])
            nc.sync.dma_start(out=wvt0[:], in_=wvt[0:128, :])
            nc.sync.dma_start(out=wvt1[:], in_=wvt[128:256, :])

            xfc0 = sg.tile([128, CHUNK], bf16, name="xfc0")
            xfc1 = sg.tile([128, CHUNK], bf16, name="xfc1")
            nc.sync.dma_start(out=xfc0[:], in_=xfc[0:128, :])
            nc.sync.dma_start(out=xfc1[:], in_=xfc[128:256, :])

            xf0 = sg.tile([128, N], bf16, name="xf0")
            xf1 = sg.tile([128, N], bf16, name="xf1")
            for s in range(0, N, 512):
                nc.sync.dma_start(out=xf0[:, s:s + 512], in_=xf[0:128, s:s + 512])
                nc.sync.dma_start(out=xf1[:, s:s + 512], in_=xf[128:256, s:s + 512])
            nc.sync.dma_start(out=ident_s[:], in_=ident[:])

            xt_s = sg.tile([128, NJT * C], f32r, name="xt_s")
            for nt in range(NJT):
                nc.gpsimd.dma_start(out=xt_s[:, nt * C:(nt + 1) * C],
                                    in_=xt[nt * 128:(nt + 1) * 128, :])
            x2tc_s = sg.tile([128, 8 * C], f32, name="x2tc_s")
            for ns in range(8):
                nc.gpsimd.dma_start(out=x2tc_s[:, ns * C:(ns + 1) * C],
                                    in_=x2tc[ns * 128:(ns + 1) * 128, :])

            q_s = sg.tile([CQK, CHUNK], bf16, name="q_s")
            k_s = sg.tile([CQK, N], bf16, name="k_s")
            vt_s = sg.tile([128, NJT * CV], bf16, name="vt_s")
            vt_view = vt_s.rearrange("p (j c) -> p j c", c=CV)
            ones_st = sg.tile([128, 2 * NJT], f32, name="ones_st")
            nc.vector.memset(ones_st[:], 1.0)
            nc.vector.tensor_copy(
                vt_view[:, :, C:CV],
                ones_st.rearrange("p (j t) -> p j t", t=2))

            camg = sg.tile([128, 8 * C], f32, name="camg")

            # ---- projections: q (chunk), k + vT (full, interleaved) ----
            for s in range(0, CHUNK, 512):
                pq = ps.tile([CQK, 512], f32, name="pq", tag="pamB0", bufs=1)
                nc.tensor.matmul(pq[:], wqt0[:], xfc0[:, s:s + 512],
                                 start=True, stop=False)
                nc.tensor.matmul(pq[:], wqt1[:], xfc1[:, s:s + 512],
                                 start=False, stop=True)
                nc.vector.tensor_scalar_add(q_s[:, s:s + 512], pq[:], bq_s[:])

            for s in range(0, N, 512):
                pk = ps.tile([CQK, 512], f32, name="pk", tag="pamB1", bufs=1)
                nc.tensor.matmul(pk[:], wkt0[:], xf0[:, s:s + 512],
                                 start=True, stop=False)
                nc.tensor.matmul(pk[:], wkt1[:], xf1[:, s:s + 512],
                                 start=False, stop=True)
                nc.vector.tensor_scalar_add(k_s[:, s:s + 512], pk[:], bk_s[:])
                for nt in range(s // 128, s // 128 + 4):
                    pv = ps.tile([128, C], f32, name="pv", tag="ec", bufs=2)
                    nc.tensor.matmul(pv[:], xf0[:, nt * 128:(nt + 1) * 128],
                                     wvt0[:], start=True, stop=False)
                    nc.tensor.matmul(pv[:], xf1[:, nt * 128:(nt + 1) * 128],
                                     wvt1[:], start=False, stop=True)
                    nc.vector.tensor_copy(vt_s[:, nt * CV:nt * CV + C], pv[:])

            # CAM Gram accumulators — filled during PAM ib0/ib1 as PE filler
            pe0 = ps.tile([128, C], f32, name="pe0", tag="ec", bufs=2)
            pe1 = ps.tile([128, C], f32, name="pe1", tag="ec", bufs=2)

            # ---- PAM i-block: eT quad -> exp -> pam accumulate ----
            # filler(pp) emits extra PE work between the eT quad and the pam
            # matmuls to cover the exp latency (pep is single-buffered).
            def pam_block(ib, tag, filler=None):
                qb = q_s[:, ib * IBLK:(ib + 1) * IBLK]
                pam_ps = [ps.tile([128, CV], f32, name=f"pam{tag}{i}",
                                  tag=f"{tag}{i}", bufs=1)
                          for i in range(IBLK // 128)]
                for pp in range(NJT // 4):
                    pep = ps.tile([128, 4 * IBLK], f32, name="pep", tag="pep",
                                  bufs=1)
                    for jj in range(4):
                        jt = 4 * pp + jj
                        nc.tensor.matmul(pep[:, jj * IBLK:(jj + 1) * IBLK],
                                         k_s[:, jt * 128:(jt + 1) * 128], qb,
                                         start=True, stop=True)
                    sblk = sp.tile([128, 4 * IBLK], bf16, name="sblk", tag="sblk")
                    nc.scalar.activation(sblk[:], pep[:], Act.Exp)
                    if filler is not None:
                        filler(pp)
                    for jj in range(4):
                        jt = 4 * pp + jj
                        vtt = vt_s[:, jt * CV:(jt + 1) * CV]
                        for isub in range(IBLK // 128):
                            nc.tensor.matmul(
                                pam_ps[isub],
                                sblk[:, jj * IBLK + isub * 128:
                                     jj * IBLK + (isub + 1) * 128],
                                vtt,
                                start=(pp == 0 and jj == 0),
                                stop=(pp == NJT // 4 - 1 and jj == 3))
                return pam_ps

            def cam_filler(base_nt):
                def filler(pp):
                    for nt in (base_nt + 2 * pp, base_nt + 2 * pp + 1):
                        xt_t = xt_s[:, nt * C:(nt + 1) * C]
                        nc.tensor.matmul(pe0[:], xt_t[:, 0:128], xt_t,
                                         start=(nt == 0), stop=(nt == NJT - 1))
                        nc.tensor.matmul(pe1[:], xt_t[:, 128:256], xt_t,
                                         start=(nt == 0), stop=(nt == NJT - 1))
                return filler

            def combine(ib, pam_ps):
                for isub in range(IBLK // 128):
                    ig = ib * (IBLK // 128) + isub
                    r = misc.tile([128, 1], f32, name="r", tag="r")
                    nc.vector.reciprocal(r[:], pam_ps[isub][:, C:C + 1])
                    t = misc.tile([128, C], f32, name="t", tag="t")
                    nc.vector.tensor_scalar(t[:], pam_ps[isub][:, 0:C],
                                            r[:], gp, op0=Alu.mult, op1=Alu.mult)
                    nc.vector.tensor_add(t[:], t[:], camg[:, ig * C:(ig + 1) * C])
                    nc.sync.dma_start(out=out[ig * 128:(ig + 1) * 128, :],
                                      in_=t[:])

            pam0 = pam_block(0, "pamA", cam_filler(0))
            pam1 = pam_block(1, "pamB", cam_filler(16))

            # ---- CAM softmax + attn transpose + cam out ----
            attn0 = sg.tile([128, C], f32, name="attn0")
            attn1 = sg.tile([128, C], f32, name="attn1")
            attnT0 = sg.tile([128, C], bf16, name="attnT0")
            attnT1 = sg.tile([128, C], bf16, name="attnT1")
            for half, pe in ((0, pe0), (1, pe1)):
                mn = misc.tile([128, 1], f32, name=f"mn{half}", tag="mn")
                sm = misc.tile([128, 1], f32, name=f"sm{half}", tag="sm")
                rs = misc.tile([128, 1], f32, name=f"rs{half}", tag="rs")
                ex = misc.tile([128, C], f32, name=f"ex{half}", tag="ex")
                nc.vector.tensor_reduce(mn[:], pe[:], axis=mybir.AxisListType.X,
                                        op=Alu.min)
                nc.scalar.activation(ex[:], pe[:], Act.Exp,
                                     bias=mn[:], scale=-1.0, accum_out=sm[:])
                nc.vector.reciprocal(rs[:], sm[:])
                dst = attn0 if half == 0 else attn1
                nc.vector.tensor_scalar_mul(dst[:], ex[:], rs[:])

            for (src, c0, dst, d0) in (
                (attn0, 0, attnT0, 0), (attn1, 0, attnT0, 128),
                (attn0, 128, attnT1, 0), (attn1, 128, attnT1, 128),
            ):
                pt = ps.tile([128, 128], f32, name="pt", tag="ec", bufs=2)
                nc.tensor.transpose(pt[:], src[:, c0:c0 + 128], ident_s[:])
                nc.vector.tensor_copy(dst[:, d0:d0 + 128], pt[:])

            # camT scaled by gamma_c, plus (2*xT + gamma_p*bv) base
            for ns in range(8):
                pc = ps.tile([128, C], f32, name="pc", tag="ec", bufs=2)
                nc.tensor.matmul(pc[:], xfc0[:, ns * 128:(ns + 1) * 128],
                                 attnT0[:], start=True, stop=False)
                nc.tensor.matmul(pc[:], xfc1[:, ns * 128:(ns + 1) * 128],
                                 attnT1[:], start=False, stop=True)
                cg = camg[:, ns * C:(ns + 1) * C]
                nc.vector.tensor_scalar(cg, pc[:], gc, None, op0=Alu.mult)
                nc.vector.tensor_add(cg, cg, x2tc_s[:, ns * C:(ns + 1) * C])

            combine(0, pam0)
            combine(1, pam1)
            pam2 = pam_block(2, "pamA")
            combine(2, pam2)
            pam3 = pam_block(3, "pamB")
            combine(3, pam3)
    nc.compile()
    return nc


_CACHE = {}


def _ensure_ntff_hook():
    """Install the axon NTFF profiling hook if the image's antenv lacks it."""
    try:
        from antenv.axon_hooks import get_axon_ntff_profile_hook  # noqa: F401
        return
    except ImportError:
        pass
    import types

    import antenv
    from trn_agent_boot.trn_boot import _ntff_profile_via_ctypes

    hook = _ntff_profile_via_ctypes("/opt/axon/libaxon_pjrt.so")
    mod = types.ModuleType("antenv.axon_hooks")
    mod.get_axon_ntff_profile_hook = lambda: hook
    mod.set_axon_ntff_profile_hook = lambda h: None
    sys.modules["antenv.axon_hooks"] = mod
    antenv.axon_hooks = mod


def kernel_run(inputs, trace=False):
    """Run on 8 cores; returns (full_output, BassKernelResults)."""
    from concourse.bass_utils import run_bass_kernel_spmd

    if trace:
        _ensure_ntff_hook()

    x = np.ascontiguousarray(np.asarray(inputs["x"], dtype=np.float32))
    wq = np.asarray(inputs["wq"], dtype=np.float32)
    wk = np.asarray(inputs["wk"], dtype=np.float32)
    wv = np.asarray(inputs["wv"], dtype=np.float32)
    bq = np.asarray(inputs["bq"], dtype=np.float32)
    bk = np.asarray(inputs["bk"], dtype=np.float32)
    bv = np.asarray(inputs["bv"], dtype=np.float32)
    gp = float(np.asarray(inputs["gamma_pam"]).reshape(-1)[0])
    gc = float(np.asarray(inputs["gamma_cam"]).reshape(-1)[0])

    key = (gp, gc)
    if key not in _CACHE:
        _CACHE[key] = _build(gp, gc)
    nc = _CACHE[key]

    import ml_dtypes

    bf = ml_dtypes.bfloat16
    xf = x.reshape(B, C, N)
    xfb = np.ascontiguousarray(xf.astype(bf))
    xtb = np.ascontiguousarray(xf.transpose(0, 2, 1))  # (B, N, C)
    wqt = np.ascontiguousarray(wq.T.astype(bf))
    wkt = np.ascontiguousarray(wk.T.astype(bf))
    wvt = np.ascontiguousarray(wv.T.astype(bf))
    ident = np.eye(128, dtype=np.float32)
    base_bv = (gp * bv).astype(np.float32)  # folded into x2tc

    in_maps = []
    for core in range(NCORES):
        g, cix = divmod(core, GROUP)
        sl = slice(cix * CHUNK, (cix + 1) * CHUNK)
        in_maps.append({
            "xf": xfb[g],
            "xfc": np.ascontiguousarray(xfb[g][:, sl]),
            "xt": xtb[g],
            "x2tc": np.ascontiguousarray(2.0 * xtb[g][sl] + base_bv[None, :]),
            "wqt": wqt, "wkt": wkt, "wvt": wvt,
            "bq": bq.reshape(CQK, 1), "bk": bk.reshape(CQK, 1),
            "ident": ident,
        })

    res = run_bass_kernel_spmd(nc, in_maps, list(range(NCORES)), trace=trace)
    outf = np.empty((B, C, N), dtype=np.float32)
    for core in range(NCORES):
        g, cix = divmod(core, GROUP)
        sl = slice(cix * CHUNK, (cix + 1) * CHUNK)
        outf[g][:, sl] = res.results[core]["out"].T
    return outf.reshape(B, C, 16, 16, 16), res


def kernel(**inputs) -> np.ndarray:
    out, _ = kernel_run(inputs)
    return out


# revision 30
# speedup vs baseline: 1.3696x; 1.3696x over previous
"""DA-Block (dual attention: PAM + CAM) Trainium2 Bass kernel.

Sharding: data-parallel over batch B (2 groups of 4 cores); within a group,
PAM's N x N attention is sharded row-wise over query voxels (1024 per core).
The small C x C CAM affinity is computed replicated on every core.

Per-core math (batch b, query chunk Q of 1024 voxels, N=4096, C=256, CQK=32):
  q = wq @ x[:, Q]            (32, 1024)
  k = wk @ x                  (32, 4096)
  vT = x^T @ wv^T             (4096, 256)   [no bias; bias folded into x2tc]
  eT[j, i] = sum_o k[o,j] q[o,i]            (PSUM, streamed over i-blocks)
  sT = exp(eT)                              (no max-sub; |e| < ~45 is fp32-safe)
  pam'[i, c'] = sum_j sT[j,i] * [vT | 1][j, c']   (c'=256 column gives denom d_i)
  camE = x @ x^T (replicated, fp32r), attn_c = softmax via exp(rowmin - e)
  camT[n, c] = sum_j attn_c[c,j] x[j, n]  for n in Q
  base[n, c] = gamma_c * camT[n, c] + (2 * xT[n, c] + gamma_p * bv[c])
  out[n, c] = gamma_p * pam'[n, c] / d_n + base[n, c]
Host gathers the (1024, 256) per-core outputs, transposes, reshapes.

Matmul dtypes: bf16 for projections and PAM attention (errors enter linearly
or through a ~0.03-abs energy perturbation that averages out in the softmax
mean); fp32r for the CAM Gram matrix, whose +-4300 energies would amplify
bf16 rounding through exp catastrophically.

PAM pipeline: the exp of an eT quad has ~1.3us latency on ScalarE; the PE
stream is software-pipelined two quads deep (pep pool bufs=3) so eT(pp+2)
runs while exp(pp) is in flight and pam(pp) never stalls.
"""

import sys

import numpy as np

for _p in ("/opt/trn_rl_repo", "/root/.axon_site/_ro/trn_rl_repo"):
    if _p not in sys.path:
        sys.path.append(_p)

B, C, CQK = 2, 256, 32
N = 4096  # 16*16*16
NCORES = 8
GROUP = 4  # cores per batch
CHUNK = N // GROUP  # 1024
IBLK = 256  # i-block (query) streaming width
NIB = CHUNK // IBLK  # 4
NJT = N // 128  # 32 j-tiles
NPP = NJT // 4  # 8 eT quads per i-block
CV = C + 2  # vT padded with two ones-columns (even free-dim for PSUM dst)


def _build(gp: float, gc: float, dbg: bool = False):
    """Build the per-core Bass kernel with gamma values baked as immediates."""
    import concourse.bacc as bacc
    import concourse.mybir as mybir
    from concourse import tile

    f32 = mybir.dt.float32
    f32r = mybir.dt.float32r
    bf16 = mybir.dt.bfloat16
    Alu = mybir.AluOpType
    Act = mybir.ActivationFunctionType

    nc = bacc.Bacc("TRN2", target_bir_lowering=False, debug=False, num_devices=NCORES)

    # xfp: x packed per 512-span as [rows 0:128 | rows 128:256] blocks
    xfp = nc.dram_tensor("xfp", [128, 2 * N], bf16, kind="ExternalInput").ap()
    xfcp = nc.dram_tensor("xfcp", [128, 2 * CHUNK], bf16, kind="ExternalInput").ap()
    xt = nc.dram_tensor("xt", [N, C], f32r, kind="ExternalInput").ap()
    x2tc = nc.dram_tensor("x2tc", [CHUNK, C], f32, kind="ExternalInput").ap()
    # wb: [wqt0|wqt1|wkt0|wkt1|wvt0|wvt1] ; fb: [bq|bk|ident]
    wb = nc.dram_tensor("wb", [128, 4 * CQK + 2 * C], bf16,
                        kind="ExternalInput").ap()
    fb = nc.dram_tensor("fb", [128, 2 + 128], f32, kind="ExternalInput").ap()
    out = nc.dram_tensor("out", [CHUNK, C], f32, kind="ExternalOutput").ap()

    with tile.TileContext(nc) as tc:
        with (
            tc.tile_pool(name="singles", bufs=1) as sg,
            tc.tile_pool(name="sp", bufs=4) as sp,
            tc.tile_pool(name="misc", bufs=4) as misc,
            tc.tile_pool(name="ps", bufs=1, space="PSUM") as ps,
        ):
            # ---- constant / persistent SBUF tiles + DMAs ----
            wb_s = sg.tile([128, 4 * CQK + 2 * C], bf16, name="wb_s")
            fb_s = sg.tile([128, 2 + 128], f32, name="fb_s")
            nc.sync.dma_start(out=wb_s[:], in_=wb[:])
            nc.sync.dma_start(out=fb_s[:], in_=fb[:])
            wqt0, wqt1 = wb_s[:, 0:CQK], wb_s[:, CQK:2 * CQK]
            wkt0, wkt1 = wb_s[:, 2 * CQK:3 * CQK], wb_s[:, 3 * CQK:4 * CQK]
            wvt0 = wb_s[:, 4 * CQK:4 * CQK + C]
            wvt1 = wb_s[:, 4 * CQK + C:4 * CQK + 2 * C]
            bq_s, bk_s, ident_s = fb_s[0:CQK, 0:1], fb_s[0:CQK, 1:2], fb_s[:, 2:130]

            xfc_s = sg.tile([128, 2 * CHUNK], bf16, name="xfc_s")
            nc.sync.dma_start(out=xfc_s[:], in_=xfcp[:])
            xfc0, xfc1 = xfc_s[:, 0:CHUNK], xfc_s[:, CHUNK:2 * CHUNK]

            xf_s = sg.tile([128, 2 * N], bf16, name="xf_s")
            for s in range(8):
                nc.sync.dma_start(out=xf_s[:, s * 1024:(s + 1) * 1024],
                                  in_=xfp[:, s * 1024:(s + 1) * 1024])

            def xfh(half, a, b):  # xf[half*128:(half+1)*128, a:b] for b-a<=512
                s = a // 512
                off = 1024 * s + 512 * half
                return xf_s[:, off + a - 512 * s:off + b - 512 * s]

            xt_s = sg.tile([128, NJT * C], f32r, name="xt_s")
            for nt in range(NJT):
                nc.gpsimd.dma_start(out=xt_s[:, nt * C:(nt + 1) * C],
                                    in_=xt[nt * 128:(nt + 1) * 128, :])
            x2tc_s = sg.tile([128, 8 * C], f32, name="x2tc_s")
            for ns in range(8):
                nc.gpsimd.dma_start(out=x2tc_s[:, ns * C:(ns + 1) * C],
                                    in_=x2tc[ns * 128:(ns + 1) * 128, :])

            q_s = sg.tile([CQK, CHUNK], bf16, name="q_s")
            k_s = sg.tile([CQK, N], bf16, name="k_s")
            vt_s = sg.tile([128, NJT * CV], bf16, name="vt_s")
            vt_view = vt_s.rearrange("p (j c) -> p j c", c=CV)
            ones_st = sg.tile([128, 2 * NJT], f32, name="ones_st")
            nc.vector.memset(ones_st[:], 1.0)
            nc.vector.tensor_copy(
                vt_view[:, :, C:CV],
                ones_st.rearrange("p (j t) -> p j t", t=2))

            camg = sg.tile([128, 8 * C], f32, name="camg")

            # ---- projections: q (chunk), k + vT (full, interleaved) ----
            for s in range(0, CHUNK, 512):
                pq = ps.tile([CQK, 512], f32, name="pq", tag="pam0", bufs=1)
                nc.tensor.matmul(pq[:], wqt0, xfc0[:, s:s + 512],
                                 start=True, stop=False)
                nc.tensor.matmul(pq[:], wqt1, xfc1[:, s:s + 512],
                                 start=False, stop=True)
                nc.vector.tensor_scalar_add(q_s[:, s:s + 512], pq[:], bq_s)

            for s in range(0, N, 512):
                pk = ps.tile([CQK, 512], f32, name="pk", tag="pam1", bufs=1)
                nc.tensor.matmul(pk[:], wkt0, xfh(0, s, s + 512),
                                 start=True, stop=False)
                nc.tensor.matmul(pk[:], wkt1, xfh(1, s, s + 512),
                                 start=False, stop=True)
                nc.vector.tensor_scalar_add(k_s[:, s:s + 512], pk[:], bk_s)
                for nt in range(s // 128, s // 128 + 4):
                    pv = ps.tile([128, C], f32, name="pv", tag="pep", bufs=3)
                    nc.tensor.matmul(pv[:], xfh(0, nt * 128, (nt + 1) * 128),
                                     wvt0, start=True, stop=False)
                    nc.tensor.matmul(pv[:], xfh(1, nt * 128, (nt + 1) * 128),
                                     wvt1, start=False, stop=True)
                    nc.vector.tensor_copy(vt_s[:, nt * CV:nt * CV + C], pv[:])

            # ---- PAM pipeline pieces ----
            pam_ps = {}
            sblks = {}

            def eT_quad(ib, pp):
                qb = q_s[:, ib * IBLK:(ib + 1) * IBLK]
                pep = ps.tile([128, 4 * IBLK], f32, name="pep", tag="pep", bufs=3)
                for jj in range(4):
                    jt = 4 * pp + jj
                    nc.tensor.matmul(pep[:, jj * IBLK:(jj + 1) * IBLK],
                                     k_s[:, jt * 128:(jt + 1) * 128], qb,
                                     start=True, stop=True)
                sblk = sp.tile([128, 4 * IBLK], bf16, name="sblk", tag="sblk")
                nc.scalar.activation(sblk[:], pep[:], Act.Exp)
                sblks[(ib, pp)] = sblk

            def pam_quad(ib, pp):
                if pp == 0:
                    pam_ps[ib] = [
                        ps.tile([128, CV], f32, name=f"pam{ib}_{i}",
                                tag=f"pam{i}", bufs=1)
                        for i in range(IBLK // 128)]
                sblk = sblks.pop((ib, pp))
                for jj in range(4):
                    jt = 4 * pp + jj
                    vtt = vt_s[:, jt * CV:(jt + 1) * CV]
                    for isub in range(IBLK // 128):
                        nc.tensor.matmul(
                            pam_ps[ib][isub],
                            sblk[:, jj * IBLK + isub * 128:
                                 jj * IBLK + (isub + 1) * 128],
                            vtt,
                            start=(pp == 0 and jj == 0),
                            stop=(pp == NPP - 1 and jj == 3))

            def pam_block(ib):
                eT_quad(ib, 0)
                eT_quad(ib, 1)
                for pp in range(NPP):
                    if pp + 2 < NPP:
                        eT_quad(ib, pp + 2)
                    pam_quad(ib, pp)

            def combine_pam(ib):
                """Stage 1: normalize+scale PAM into SBUF, freeing PSUM."""
                ts = []
                for isub in range(IBLK // 128):
                    r = misc.tile([128, 1], f32, name="r", tag="r")
                    nc.vector.reciprocal(r[:], pam_ps[ib][isub][:, C:C + 1])
                    t = misc.tile([128, C], f32, name="t", tag="t")
                    nc.vector.tensor_scalar(t[:], pam_ps[ib][isub][:, 0:C],
                                            r[:], gp, op0=Alu.mult, op1=Alu.mult)
                    ts.append(t)
                return ts

            def combine_out(ib, ts):
                """Stage 2: add CAM+residual base, store."""
                for isub in range(IBLK // 128):
                    ig = ib * (IBLK // 128) + isub
                    t = ts[isub]
                    nc.vector.tensor_add(t[:], t[:], camg[:, ig * C:(ig + 1) * C])
                    nc.sync.dma_start(out=out[ig * 128:(ig + 1) * 128, :],
                                      in_=t[:])

            pam_block(0)
            t0 = combine_pam(0)

            # ---- CAM (replicated): Gram accumulated via SBUF partials ----
            # NOTE: each accumulation group needs its own PSUM bank — a
            # matmul with start=True clears has_written for the whole bank,
            # so two interleaved accumulations must not share one.
            ecacc = sg.tile([128, 2 * C], f32, name="ecacc")
            for g in range(4):
                peg0 = ps.tile([128, C], f32, name="peg0", tag="pep", bufs=3)
                peg1 = ps.tile([128, C], f32, name="peg1", tag="pep", bufs=3)
                for nt in range(8 * g, 8 * g + 8):
                    xt_t = xt_s[:, nt * C:(nt + 1) * C]
                    nc.tensor.matmul(peg0[:], xt_t[:, 0:128], xt_t,
                                     start=(nt == 8 * g), stop=(nt == 8 * g + 7))
                    nc.tensor.matmul(peg1[:], xt_t[:, 128:256], xt_t,
                                     start=(nt == 8 * g), stop=(nt == 8 * g + 7))
                if g == 0:
                    nc.vector.tensor_copy(ecacc[:, 0:C], peg0[:])
                    nc.vector.tensor_copy(ecacc[:, C:2 * C], peg1[:])
                else:
                    nc.vector.tensor_add(ecacc[:, 0:C], ecacc[:, 0:C], peg0[:])
                    nc.vector.tensor_add(ecacc[:, C:2 * C], ecacc[:, C:2 * C],
                                         peg1[:])

            attn0 = sg.tile([128, C], f32, name="attn0")
            attn1 = sg.tile([128, C], f32, name="attn1")
            attnT0 = sg.tile([128, C], bf16, name="attnT0")
            attnT1 = sg.tile([128, C], bf16, name="attnT1")
            for half in (0, 1):
                pe = ecacc[:, half * C:(half + 1) * C]
                mn = misc.tile([128, 1], f32, name=f"mn{half}", tag="mn")
                sm = misc.tile([128, 1], f32, name=f"sm{half}", tag="sm")
                rs = misc.tile([128, 1], f32, name=f"rs{half}", tag="rs")
                ex = misc.tile([128, C], f32, name=f"ex{half}", tag="ex")
                nc.vector.tensor_reduce(mn[:], pe, axis=mybir.AxisListType.X,
                                        op=Alu.min)
                nc.scalar.activation(ex[:], pe, Act.Exp,
                                     bias=mn[:], scale=-1.0, accum_out=sm[:])
                nc.vector.reciprocal(rs[:], sm[:])
                dst = attn0 if half == 0 else attn1
                nc.vector.tensor_scalar_mul(dst[:], ex[:], rs[:])

            # prefetch first two eT quads of ib1 under the softmax latency
            eT_quad(1, 0)
            eT_quad(1, 1)

            for (src, c0, dst, d0) in (
                (attn0, 0, attnT0, 0), (attn1, 0, attnT0, 128),
                (attn0, 128, attnT1, 0), (attn1, 128, attnT1, 128),
            ):
                pt = ps.tile([128, 128], f32, name="pt", tag="pep", bufs=3)
                nc.tensor.transpose(pt[:], src[:, c0:c0 + 128], ident_s)
                nc.vector.tensor_copy(dst[:, d0:d0 + 128], pt[:])

            # camT scaled by gamma_c, plus (2*xT + gamma_p*bv) base
            for ns in range(8):
                pc = ps.tile([128, C], f32, name="pc", tag="pep", bufs=3)
                nc.tensor.matmul(pc[:], xfc0[:, ns * 128:(ns + 1) * 128],
                                 attnT0[:], start=True, stop=False)
                nc.tensor.matmul(pc[:], xfc1[:, ns * 128:(ns + 1) * 128],
                                 attnT1[:], start=False, stop=True)
                cg = camg[:, ns * C:(ns + 1) * C]
                nc.vector.tensor_scalar(cg, pc[:], gc, None, op0=Alu.mult)
                nc.vector.tensor_add(cg, cg, x2tc_s[:, ns * C:(ns + 1) * C])

            combine_out(0, t0)

            # ib1: quads 0,1 already emitted above
            for pp in range(NPP):
                if pp + 2 < NPP:
                    eT_quad(1, pp + 2)
                pam_quad(1, pp)
            combine_out(1, combine_pam(1))

            for ib in (2, 3):
                pam_block(ib)
                combine_out(ib, combine_pam(ib))

            if dbg:
                d_ecacc = nc.dram_tensor("d_ecacc", [128, 2 * C], f32,
                                         kind="ExternalOutput").ap()
                d_attn = nc.dram_tensor("d_attn", [128, 2 * C], f32,
                                        kind="ExternalOutput").ap()
                d_attnT = nc.dram_tensor("d_attnT", [128, 2 * C], f32,
                                         kind="ExternalOutput").ap()
                d_camg = nc.dram_tensor("d_camg", [128, 8 * C], f32,
                                        kind="ExternalOutput").ap()
                d_q = nc.dram_tensor("d_q", [CQK, CHUNK], f32,
                                     kind="ExternalOutput").ap()
                d_k = nc.dram_tensor("d_k", [CQK, N], f32,
                                     kind="ExternalOutput").ap()
                d_vt = nc.dram_tensor("d_vt", [128, NJT * CV], f32,
                                      kind="ExternalOutput").ap()
                nc.sync.dma_start(out=d_ecacc[:], in_=ecacc[:])
                nc.sync.dma_start(out=d_camg[:], in_=camg[:])
                cv32 = sg.tile([128, 2 * C], f32, name="cv32")
                nc.vector.tensor_copy(cv32[:, 0:C], attn0[:])
                nc.vector.tensor_copy(cv32[:, C:2 * C], attn1[:])
                nc.sync.dma_start(out=d_attn[:], in_=cv32[:])
                cv33 = sg.tile([128, 2 * C], f32, name="cv33")
                nc.vector.tensor_copy(cv33[:, 0:C], attnT0[:])
                nc.vector.tensor_copy(cv33[:, C:2 * C], attnT1[:])
                nc.sync.dma_start(out=d_attnT[:], in_=cv33[:])
                cq = sg.tile([CQK, CHUNK], f32, name="cq")
                nc.vector.tensor_copy(cq[:], q_s[:])
                nc.sync.dma_start(out=d_q[:], in_=cq[:])
                ck = sg.tile([CQK, N], f32, name="ck")
                nc.vector.tensor_copy(ck[:], k_s[:])
                nc.sync.dma_start(out=d_k[:], in_=ck[:])
                cvt = sg.tile([128, NJT * CV], f32, name="cvt")
                nc.vector.tensor_copy(cvt[:], vt_s[:])
                nc.sync.dma_start(out=d_vt[:], in_=cvt[:])
    nc.compile()
    return nc


_CACHE = {}


def _ensure_ntff_hook():
    """Install the axon NTFF profiling hook if the image's antenv lacks it."""
    try:
        from antenv.axon_hooks import get_axon_ntff_profile_hook  # noqa: F401
        return
    except ImportError:
        pass
    import types

    import antenv
    from trn_agent_boot.trn_boot import _ntff_profile_via_ctypes

    hook = _ntff_profile_via_ctypes("/opt/axon/libaxon_pjrt.so")
    mod = types.ModuleType("antenv.axon_hooks")
    mod.get_axon_ntff_profile_hook = lambda: hook
    mod.set_axon_ntff_profile_hook = lambda h: None
    sys.modules["antenv.axon_hooks"] = mod
    antenv.axon_hooks = mod


def kernel_run(inputs, trace=False):
    """Run on 8 cores; returns (full_output, BassKernelResults)."""
    from concourse.bass_utils import run_bass_kernel_spmd

    if trace:
        _ensure_ntff_hook()

    x = np.ascontiguousarray(np.asarray(inputs["x"], dtype=np.float32))
    wq = np.asarray(inputs["wq"], dtype=np.float32)
    wk = np.asarray(inputs["wk"], dtype=np.float32)
    wv = np.asarray(inputs["wv"], dtype=np.float32)
    bq = np.asarray(inputs["bq"], dtype=np.float32)
    bk = np.asarray(inputs["bk"], dtype=np.float32)
    bv = np.asarray(inputs["bv"], dtype=np.float32)
    gp = float(np.asarray(inputs["gamma_pam"]).reshape(-1)[0])
    gc = float(np.asarray(inputs["gamma_cam"]).reshape(-1)[0])

    key = (gp, gc)
    if key not in _CACHE:
        _CACHE[key] = _build(gp, gc)
    nc = _CACHE[key]

    import ml_dtypes

    bf = ml_dtypes.bfloat16
    xf = x.reshape(B, C, N)
    xfb = xf.astype(bf)
    xtb = np.ascontiguousarray(xf.transpose(0, 2, 1))  # (B, N, C)

    # weight blob [wqt0|wqt1|wkt0|wkt1|wvt0|wvt1], all (128, .) bf16
    wqt, wkt, wvt = wq.T.astype(bf), wk.T.astype(bf), wv.T.astype(bf)
    wb = np.concatenate([wqt[0:128], wqt[128:256], wkt[0:128], wkt[128:256],
                         wvt[0:128], wvt[128:256]], axis=1)
    wb = np.ascontiguousarray(wb)
    # f32 blob [bq|bk|ident] (128, 130)
    fb = np.zeros((128, 130), dtype=np.float32)
    fb[0:CQK, 0] = bq
    fb[0:CQK, 1] = bk
    fb[:, 2:130] = np.eye(128, dtype=np.float32)

    def pack_spans(a):  # (256, M) -> (128, 2M) span-interleaved
        m = a.shape[1]
        blocks = []
        for s in range(0, m, 512):
            blocks.append(a[0:128, s:s + 512])
            blocks.append(a[128:256, s:s + 512])
        return np.ascontiguousarray(np.concatenate(blocks, axis=1))

    base_bv = (gp * bv).astype(np.float32)  # folded into x2tc

    in_maps = []
    for core in range(NCORES):
        g, cix = divmod(core, GROUP)
        sl = slice(cix * CHUNK, (cix + 1) * CHUNK)
        in_maps.append({
            "xfp": pack_spans(xfb[g]),
            "xfcp": np.concatenate([xfb[g][0:128, sl], xfb[g][128:256, sl]],
                                   axis=1).copy(),
            "xt": xtb[g],
            "x2tc": np.ascontiguousarray(2.0 * xtb[g][sl] + base_bv[None, :]),
            "wb": wb, "fb": fb,
        })

    res = run_bass_kernel_spmd(nc, in_maps, list(range(NCORES)), trace=trace)
    outf = np.empty((B, C, N), dtype=np.float32)
    for core in range(NCORES):
        g, cix = divmod(core, GROUP)
        sl = slice(cix * CHUNK, (cix + 1) * CHUNK)
        outf[g][:, sl] = res.results[core]["out"].T
    return outf.reshape(B, C, 16, 16, 16), res


def kernel(**inputs) -> np.ndarray:
    out, _ = kernel_run(inputs)
    return out
